# revision 7
# baseline (speedup 1.0000x reference)
"""Bahdanau (additive MLP) attention on 8 Trainium2 NeuronCores.

reference:
    q = query @ Wq.T            [B,M,H]
    k = memory @ Wm.T           [B,N,H]
    aligns[b,m,n] = w_out . tanh(q[b,m,:] + k[b,n,:])
    scores = softmax(aligns, axis=-1)
    out = scores @ memory       [B,M,D]

Strategy (v2, sine factorization): instead of materializing the
[B,M,N,H] tanh on the scalar engine (33.5M tanh/core ~= 218us floor,
what the previous kernel did), approximate

    tanh(x) ~= sum_f a_f sin(f*w1*x),   f in {1,2,3,4,6,8}, w1 = pi/L

and use sin(f*w1*(q+m)) = sin(f*w1*q)cos(f*w1*m) + cos(f*w1*q)sin(f*w1*m):
the (m,n) reduction over h becomes a bf16 PE matmul with contraction
dim (f, h) = 12*512, i.e. 48 [128x128]x[128x512] matmuls per core
(~10us at 2.4GHz) instead of 218us of scalar-engine tanh.

Per-core plane production:
  - ACT computes sin/cos for f in {1,2,3} (q-side from the q-projection
    PSUM, m-side from an SBUF copy), with cos = sin(. + pi/2).
  - DVE derives f in {4,6,8} by angle doubling with scale bookkeeping:
    stored sigma_f = sin_f / 2^d stays a pure tensor_tensor product
    chain (2x DVE mode), t_f = sigma_{f/2}^2 substitutes for cos_f
    (cos_f = 1 - 2^g t_f); the affine constant is per-m only and drops
    out of the softmax. True cos (tensor_scalar 1-2^g*t) is materialized
    only where a chain or a q-side fold needs it.
  - w_out (sign-folded into Wq/Wm rows so w>=0, h sorted by w) and the
    coefficients a_f are folded into the q-side planes via host-built
    bf16 masks (exact per (c,hp) values).
Softmax epilogue: exp with fused row-sum accumulator, scores scaled by
1/s before 4 PE transposes, output matmul accumulates scoresT_j @ mem_j,
DMA straight from PSUM.

Sharding: core i handles batch b = i//2 and M-half i%2 (128 query rows).
Fully data-parallel -- softmax over N is local to a core. No collectives.

Numerics (numpy emulation of the exact device plane algebra, bf16
rounding at every step): rel err 0.0095 vs the f32 reference (gate 2e-2).
"""

import numpy as np

import concourse.tile as tile
from concourse import bacc, mybir
from concourse.alu_op_type import AluOpType
from concourse.bass_utils import run_bass_kernel_spmd

f32 = mybir.dt.float32
bf16 = mybir.dt.bfloat16
AF = mybir.ActivationFunctionType
MULT = AluOpType.mult
ADD = AluOpType.add

B, M, N, D, H = 4, 256, 512, 512, 512
NCORES = 8
ML = M * B // NCORES  # 128 query rows per core

# sine-series approximation of tanh on [-L, L] (least squares, gaussian
# weight matching the q+m distribution + uniform floor; see module doc)
L_RANGE = 10.4
FREQS = (1, 2, 3, 4, 6, 8)
A_COEF = (1.24090491, -0.0026346, 0.26740208, 0.09456087, 0.08249926, 0.02866788)
W1 = np.pi / L_RANGE

# masks folded into the q side: (name, coefficient multiplier)
# pairing: A1_f (sin-fold) x {cos-plane or t-plane}; A2_f (cos-fold) x
# {sin-plane or sigma-plane}. Derived freqs use sigma/t planes:
#   f=4: sin4 = 2*sq4, cos4 = 1-2*tm4  -> alphas (-4a4, 2a4)
#   f=6: same structure from f=3       -> (-4a6, 2a6)
#   f=8: sin8 = 4*sq8, cos8 = 1-8*tm8  -> (-32a8, 4a8)
MASKS = (
    ("1", A_COEF[0]), ("2", A_COEF[1]), ("3", A_COEF[2]),
    ("4s", -4 * A_COEF[3]), ("4c", 2 * A_COEF[3]),
    ("6s", -4 * A_COEF[4]), ("6c", 2 * A_COEF[4]),
    ("8s", -32 * A_COEF[5]), ("8c", 4 * A_COEF[5]),
)
MIDX = {name: i for i, (name, _) in enumerate(MASKS)}

_HINTS = (
    mybir.EngineType.PE,
    mybir.EngineType.Activation,
    mybir.EngineType.DVE,
    mybir.EngineType.SP,
    mybir.EngineType.Pool,
)


def _build():
    nc = bacc.Bacc("TRN2", target_bir_lowering=False, debug=False, num_devices=NCORES)

    # DRAM inputs, laid out partition-major by the host:
    # qT   [dp, (dc, m)]      = query[b, m0+m, dc*128+dp]           bf16
    # wqT  [dp, (dc, c, hp)]  = Wq'[c*128+hp, dc*128+dp]            bf16
    # wmT  [dp, (dc, c, hp)]  = Wm'[c*128+hp, dc*128+dp]            bf16
    # memT [dp, (dc, n)]      = memory[b, n, dc*128+dp]             bf16
    # memN [np_, (j, d)]      = memory[b, j*128+np_, d]             bf16
    # msk  [hp, (u, c, m)]    = mask_u[c*128+hp] (m-broadcast)      bf16
    # idn  [p, q]             = identity                            bf16
    qT = nc.dram_tensor("qT", [128, 512], bf16, kind="ExternalInput")
    wqT = nc.dram_tensor("wqT", [128, 2048], bf16, kind="ExternalInput")
    wmT = nc.dram_tensor("wmT", [128, 2048], bf16, kind="ExternalInput")
    memT = nc.dram_tensor("memT", [128, 2048], bf16, kind="ExternalInput")
    memN = nc.dram_tensor("memN", [128, 2048], bf16, kind="ExternalInput")
    msk = nc.dram_tensor("msk", [128, len(MASKS) * 512], bf16, kind="ExternalInput")
    idn = nc.dram_tensor("idn", [128, 128], bf16, kind="ExternalInput")
    out = nc.dram_tensor("out", [128, 512], f32, kind="ExternalOutput")

    with tile.TileContext(nc) as tc:
        with (
            tc.tile_pool(name="const", bufs=1) as const,
            tc.tile_pool(name="kp", bufs=4, space="PSUM") as kp,
            tc.tile_pool(name="qal", bufs=1, space="PSUM") as qal,
        ):
            # ---- SBUF tiles -------------------------------------------------
            qT_sb = const.tile([128, 512], bf16)
            wqT_sb = const.tile([128, 2048], bf16)
            wmT_sb = const.tile([128, 2048], bf16)
            memT_sb = const.tile([128, 2048], bf16)
            memN_sb = const.tile([128, 2048], bf16)
            msk_sb = const.tile([128, len(MASKS) * 512], bf16)
            idn_sb = const.tile([128, 128], bf16)
            warm_sb = const.tile([128, 128], bf16)
            halfpi_sb = const.tile([128, 1], f32)
            m_sb = const.tile([128, 2048], f32)  # [hp, (c, n)] f32

            # ACT planes: q-side [hp, (c,m)] 512 wide, m-side [hp,(c,n)] 2048
            qs = {f: const.tile([128, 512], bf16, name=f"qs{f}") for f in (1, 2, 3)}
            qc = {f: const.tile([128, 512], bf16, name=f"qc{f}") for f in (1, 2, 3)}
            ms = {f: const.tile([128, 2048], bf16, name=f"ms{f}") for f in (1, 2, 3)}
            mc = {f: const.tile([128, 2048], bf16, name=f"mc{f}") for f in (1, 2, 3)}
            # DVE-derived q planes
            sq4 = const.tile([128, 512], bf16, name="sq4")
            tq4 = const.tile([128, 512], bf16, name="tq4")
            cq4 = const.tile([128, 512], bf16, name="cq4")
            sq6 = const.tile([128, 512], bf16, name="sq6")
            tq6 = const.tile([128, 512], bf16, name="tq6")
            cq6 = const.tile([128, 512], bf16, name="cq6")
            sq8 = const.tile([128, 512], bf16, name="sq8")
            tq8 = const.tile([128, 512], bf16, name="tq8")
            cq8 = const.tile([128, 512], bf16, name="cq8")
            # DVE-derived m planes
            sm4 = const.tile([128, 2048], bf16, name="sm4")
            tm4 = const.tile([128, 2048], bf16, name="tm4")
            cm4 = const.tile([128, 2048], bf16, name="cm4")
            sm6 = const.tile([128, 2048], bf16, name="sm6")
            tm6 = const.tile([128, 2048], bf16, name="tm6")
            sm8 = const.tile([128, 2048], bf16, name="sm8")
            tm8 = const.tile([128, 2048], bf16, name="tm8")
            # folded q planes (12)
            A = {k: const.tile([128, 512], bf16, name=f"A{k}") for k in
                 ("1s", "1c", "2s", "2c", "3s", "3c", "4s", "4c", "6s", "6c", "8s", "8c")}

            exp_sb = const.tile([128, 512], bf16)
            expsc_sb = const.tile([128, 512], bf16)
            sums_sb = const.tile([128, 1], f32)
            rs_sb = const.tile([128, 1], f32)
            scT = [const.tile([128, 128], bf16, name=f"scT{j}") for j in range(4)]
            out_sb = const.tile([128, 512], f32)

            def mslice(name):
                u = MIDX[name]
                return msk_sb[:, u * 512:(u + 1) * 512]

            # ---- prologue: table warm + DMA + PE warm ----------------------
            nc.vector.memset(warm_sb[:], 1.0)
            nc.vector.memset(halfpi_sb[:], float(np.pi / 2))
            nc.scalar.activation(warm_sb[:, 0:1], warm_sb[:, 0:1], AF.Sin)

            nc.sync.dma_start(qT_sb[:], qT.ap())
            nc.sync.dma_start(wqT_sb[:], wqT.ap())
            for dc in range(4):
                sl = slice(dc * 512, (dc + 1) * 512)
                nc.sync.dma_start(wmT_sb[:, sl], wmT.ap()[:, sl])
                nc.sync.dma_start(memT_sb[:, sl], memT.ap()[:, sl])
            nc.sync.dma_start(msk_sb[:], msk.ap())
            nc.sync.dma_start(memN_sb[:], memN.ap())
            nc.sync.dma_start(idn_sb[:], idn.ap())

            # PE warm-up: HAM clock gate holds PE at 1.2GHz until ~3.4us busy
            warm_ps = kp.tile([128, 128], f32, tag="mp")
            for _ in range(8):
                nc.tensor.matmul(warm_ps[:], warm_sb[:], warm_sb[:],
                                 start=True, stop=True)

            # ---- projections -----------------------------------------------
            # q_projT[h, m]: qp[hp, c*128+m] for h = c*128+hp
            qp = qal.tile([128, 512], f32, tag="qal", name="qp")
            for c in range(4):
                for dc in range(4):
                    nc.tensor.matmul(
                        qp[:, c * 128:(c + 1) * 128],
                        wqT_sb[:, dc * 512 + c * 128: dc * 512 + (c + 1) * 128],
                        qT_sb[:, dc * 128:(dc + 1) * 128],
                        start=(dc == 0), stop=(dc == 3),
                    )
            # m_projT[h, n]: mp_c[hp, n] for h = c*128+hp
            mp = [kp.tile([128, 512], f32, tag="mp", name=f"mp{c}") for c in range(4)]
            for dc in range(4):
                for c in range(4):
                    nc.tensor.matmul(
                        mp[c][:],
                        wmT_sb[:, dc * 512 + c * 128: dc * 512 + (c + 1) * 128],
                        memT_sb[:, dc * 512:(dc + 1) * 512],
                        start=(dc == 0), stop=(dc == 3),
                    )
            # copies psum -> sbuf f32 so ACT m instrs can be [128, 2048]
            for c in range(4):
                nc.vector.tensor_copy(m_sb[:, c * 512:(c + 1) * 512], mp[c][:])

            # ---- ACT planes ------------------------------------------------
            for f in (1, 2, 3):
                w = float(W1 * f)
                nc.scalar.activation(qs[f][:], qp[:], AF.Sin, scale=w)
                nc.scalar.activation(qc[f][:], qp[:], AF.Sin, bias=halfpi_sb[:], scale=w)
            # m planes: f=2 first (feeds the longest DVE chain), f=1 last
            # (its planes go straight to PE, shortest tail)
            for f in (2, 3, 1):
                w = float(W1 * f)
                nc.scalar.activation(ms[f][:], m_sb[:], AF.Sin, scale=w)
                nc.scalar.activation(mc[f][:], m_sb[:], AF.Sin, bias=halfpi_sb[:], scale=w)

            # ---- DVE: q-side derivations + folds ---------------------------
            tt = nc.vector.tensor_tensor
            ts = nc.vector.tensor_scalar
            # f=4 from f=2; f=8 from f=4; f=6 from f=3 (q side, sigma = sin/2^d)
            tt(sq4[:], qs[2][:], qc[2][:], MULT)
            tt(tq4[:], qs[2][:], qs[2][:], MULT)
            ts(cq4[:], tq4[:], -2.0, 1.0, MULT, ADD)
            tt(sq6[:], qs[3][:], qc[3][:], MULT)
            tt(tq6[:], qs[3][:], qs[3][:], MULT)
            ts(cq6[:], tq6[:], -2.0, 1.0, MULT, ADD)
            tt(sq8[:], sq4[:], cq4[:], MULT)
            tt(tq8[:], sq4[:], sq4[:], MULT)
            ts(cq8[:], tq8[:], -8.0, 1.0, MULT, ADD)
            # folds (A1_f = sin-side, A2_f = cos-side)
            tt(A["1s"][:], qs[1][:], mslice("1"), MULT)
            tt(A["1c"][:], qc[1][:], mslice("1"), MULT)
            tt(A["2s"][:], qs[2][:], mslice("2"), MULT)
            tt(A["2c"][:], qc[2][:], mslice("2"), MULT)
            tt(A["3s"][:], qs[3][:], mslice("3"), MULT)
            tt(A["3c"][:], qc[3][:], mslice("3"), MULT)
            tt(A["4s"][:], sq4[:], mslice("4s"), MULT)
            tt(A["4c"][:], cq4[:], mslice("4c"), MULT)
            tt(A["6s"][:], sq6[:], mslice("6s"), MULT)
            tt(A["6c"][:], cq6[:], mslice("6c"), MULT)
            tt(A["8s"][:], sq8[:], mslice("8s"), MULT)
            tt(A["8c"][:], cq8[:], mslice("8c"), MULT)

            # ---- DVE: m-side derivations -----------------------------------
            tt(sm4[:], ms[2][:], mc[2][:], MULT)
            tt(tm4[:], ms[2][:], ms[2][:], MULT)
            ts(cm4[:], tm4[:], -2.0, 1.0, MULT, ADD)
            tt(sm8[:], sm4[:], cm4[:], MULT)
            tt(tm8[:], sm4[:], sm4[:], MULT)
            tt(sm6[:], ms[3][:], mc[3][:], MULT)
            tt(tm6[:], ms[3][:], ms[3][:], MULT)

            # ---- feature matmuls: aligns[m, n] -----------------------------
            al = qal.tile([128, 512], f32, tag="qal", name="al")
            # (A1_f, cos-ish m plane), (A2_f, sin-ish m plane); order by
            # expected plane readiness
            pairs = [
                (A["2s"], mc[2]), (A["2c"], ms[2]),
                (A["4s"], tm4), (A["4c"], sm4),
                (A["3s"], mc[3]), (A["3c"], ms[3]),
                (A["8s"], tm8), (A["8c"], sm8),
                (A["6s"], tm6), (A["6c"], sm6),
                (A["1s"], mc[1]), (A["1c"], ms[1]),
            ]
            nmm = len(pairs) * 4
            i = 0
            for Aq, Bm in pairs:
                for c in range(4):
                    nc.tensor.matmul(
                        al[:],
                        Aq[:, c * 128:(c + 1) * 128],
                        Bm[:, c * 512:(c + 1) * 512],
                        start=(i == 0), stop=(i == nmm - 1),
                    )
                    i += 1

            # ---- softmax + output ------------------------------------------
            # no max subtraction: |aligns| <= sum_u ||U_u||_1 ~ 65, e^65 fits f32
            nc.scalar.activation(exp_sb[:], al[:], AF.Exp, accum_out=sums_sb[:])
            nc.vector.reciprocal(rs_sb[:], sums_sb[:])
            nc.vector.tensor_scalar_mul(expsc_sb[:], exp_sb[:], rs_sb[:])

            o_ps = kp.tile([128, 512], f32, tag="mp", name="ops")
            tr = kp.tile([128, 512], bf16, tag="mp", name="tr")
            for j in range(4):
                nc.tensor.transpose(tr[:, j * 128:(j + 1) * 128],
                                    expsc_sb[:, j * 128:(j + 1) * 128], idn_sb[:])
                nc.vector.tensor_copy(scT[j][:], tr[:, j * 128:(j + 1) * 128])
                nc.tensor.matmul(
                    o_ps[:], scT[j][:], memN_sb[:, j * 512:(j + 1) * 512],
                    start=(j == 0), stop=(j == 3),
                )
            nc.scalar.copy(out_sb[:], o_ps[:])
            nc.sync.dma_start(out.ap(), out_sb[:])

    nc.compile()
    return nc


_nc_cache = {}


def _get_nc():
    if "nc" not in _nc_cache:
        _nc_cache["nc"] = _build()
    return _nc_cache["nc"]


def _shard_inputs(query, memory, Wq, Wm, w_out):
    import ml_dtypes

    bf = ml_dtypes.bfloat16
    query = np.ascontiguousarray(query, dtype=np.float32)
    memory = np.ascontiguousarray(memory, dtype=np.float32)
    Wq = np.ascontiguousarray(Wq, dtype=np.float32)
    Wm = np.ascontiguousarray(Wm, dtype=np.float32)
    w_out = np.ascontiguousarray(w_out, dtype=np.float32)

    # fold sign of w into Wq/Wm rows (tanh odd), sort h by |w| (cosmetic
    # but keeps mask values smooth per partition)
    sgn = np.sign(w_out)
    sgn[sgn == 0] = 1.0
    order = np.argsort(w_out * sgn)
    wtld = (w_out * sgn)[order]  # >= 0, [H]
    Wqp = (Wq * sgn[:, None])[order]
    Wmp = (Wm * sgn[:, None])[order]

    # [dp, (dc, c, hp)]
    wqT_h = np.ascontiguousarray(
        Wqp.T.reshape(4, 128, 4, 128).transpose(1, 0, 2, 3).reshape(128, 2048)
    ).astype(bf)
    wmT_h = np.ascontiguousarray(
        Wmp.T.reshape(4, 128, 4, 128).transpose(1, 0, 2, 3).reshape(128, 2048)
    ).astype(bf)

    # masks [hp, (u, c, m)]: mask_u[c*128+hp] broadcast along m
    msk_h = np.empty((128, len(MASKS) * 512), np.float32)
    for u, (_, alpha) in enumerate(MASKS):
        vals = (alpha * wtld).reshape(4, 128).T  # [hp, c]
        msk_h[:, u * 512:(u + 1) * 512] = np.repeat(vals, 128, axis=1)
    msk_h = msk_h.astype(bf)
    idn_h = np.eye(128, dtype=np.float32).astype(bf)

    in_maps = []
    for i in range(NCORES):
        b, mh = divmod(i, 2)
        qT_h = np.ascontiguousarray(
            query[b, mh * ML:(mh + 1) * ML, :]
            .T.reshape(4, 128, 128).transpose(1, 0, 2).reshape(128, 512)
        ).astype(bf)
        memT_h = np.ascontiguousarray(
            memory[b].T.reshape(4, 128, 512).transpose(1, 0, 2).reshape(128, 2048)
        ).astype(bf)
        memN_h = np.ascontiguousarray(
            memory[b].reshape(4, 128, 512).transpose(1, 0, 2).reshape(128, 2048)
        ).astype(bf)
        in_maps.append({
            "qT": qT_h, "wqT": wqT_h, "wmT": wmT_h,
            "memT": memT_h, "memN": memN_h, "msk": msk_h, "idn": idn_h,
        })
    return in_maps


def kernel(query, memory, Wq, Wm, w_out):
    nc = _get_nc()
    in_maps = _shard_inputs(query, memory, Wq, Wm, w_out)
    res = run_bass_kernel_spmd(nc, in_maps, core_ids=list(range(NCORES)))
    full = np.empty((B, M, D), dtype=np.float32)
    for i in range(NCORES):
        b, mh = divmod(i, 2)
        full[b, mh * ML:(mh + 1) * ML, :] = res.results[i]["out"]
    return full
